# revision 21
# baseline (speedup 1.0000x reference)
"""Kandinsky5Attention Bass/Tile kernel for 8 Trainium2 NeuronCores.

Sharding: core = (batch b, head-group g): 2 batches x 4 groups of 4 heads.
Each core computes q/k/v for its 512 features of its batch, attention for
its 4 heads, and a partial output projection over its 512 contraction dims.
Host sums the 4 partials per batch and adds the output bias.

v3 design:
 - fp16 matmuls for QKV + scores (full PE rate, small rounding); bf16 for
   es/v/out-proj (exp range needs bf16); fp32 PSUM accumulation
 - q/k/v and attention outputs stay SBUF-resident (no DRAM spill)
 - RMSNorm weight and 1/sqrt(HD) folded into rope coefficients on the host
 - rsqrt = ACT Sqrt + DVE reciprocal_approx_fast (no ACT table thrash)
 - per-position q scale via gpsimd partition_broadcast (base partition 0);
   k scale folded into the exp's per-partition FP32 scale vector
 - DMA queues: bulk weights on sync, x + output on vector, small
   dependent transfers (rope swap, norm rows) on gpsimd
"""
import math

import numpy as np
import ml_dtypes

import concourse.bass as bass
import concourse.mybir as mybir
import concourse.tile as tile
from concourse import bacc
from concourse.bass_utils import run_bass_kernel_spmd

B, S, C, HD = 2, 2048, 2048, 128
H = C // HD            # 16 heads
HG = 4                 # head groups (cores per batch)
HPG = H // HG          # 4 heads per group
GF = HPG * HD          # 512 features per group
EPS = float(np.finfo(np.float32).eps)
N_CORES = 8
NCC = C // 128         # 16 contraction chunks
ST = 512               # phase-1 s tile
N_ST = S // ST         # 4
SQ_T = 1024            # phase-2 query block
N_SQ = S // SQ_T       # 2
N_SK = S // 128        # 16 key chunks

F32 = mybir.dt.float32
BF16 = mybir.dt.bfloat16
FP16 = mybir.dt.float16
AF = mybir.ActivationFunctionType
ALU = mybir.AluOpType
ISCALE = 1.0 / math.sqrt(HD)


def build_program():
    nc = bacc.Bacc("TRN2", target_bir_lowering=False, debug=False,
                   num_devices=N_CORES)

    xt = nc.dram_tensor("xt", [C, S], FP16, kind="ExternalInput")
    wqt = nc.dram_tensor("wqt", [C, GF], FP16, kind="ExternalInput")
    wkt = nc.dram_tensor("wkt", [C, GF], FP16, kind="ExternalInput")
    wvt = nc.dram_tensor("wvt", [C, GF], FP16, kind="ExternalInput")
    wot = nc.dram_tensor("wot", [GF, C], BF16, kind="ExternalInput")
    bqd = nc.dram_tensor("bq", [HPG, HD], F32, kind="ExternalInput")
    bkd = nc.dram_tensor("bk", [HPG, HD], F32, kind="ExternalInput")
    bvd = nc.dram_tensor("bv", [1, GF], FP16, kind="ExternalInput")
    ropeqd = nc.dram_tensor("ropeq", [2, 128, S], FP16, kind="ExternalInput")
    ropekd = nc.dram_tensor("ropek", [2, 128, S], FP16, kind="ExternalInput")
    outd = nc.dram_tensor("out", [S, C], F32, kind="ExternalOutput")
    rkscr = nc.dram_tensor("rk_scr", [HPG, S], F32)  # transpose bounce

    xt_r = xt[:, :].rearrange("(cc p) s -> p cc s", p=128)
    wq_r = wqt[:, :].rearrange("(cc p) g -> p cc g", p=128)
    wk_r = wkt[:, :].rearrange("(cc p) g -> p cc g", p=128)
    wv_r = wvt[:, :].rearrange("(cc p) g -> p cc g", p=128)
    wo_r = wot[:, :].rearrange("(h p) c -> p h c", p=128)

    with tile.TileContext(nc) as tc, \
            nc.allow_low_precision(reason="fp16/bf16 matmuls within budget"):
        with tc.tile_pool(name="glob", bufs=1) as glob:
            ones_col16 = glob.tile([128, 1], FP16)
            nc.vector.memset(ones_col16[:], 1.0)
            ones_colb = glob.tile([128, 1], BF16)
            nc.vector.memset(ones_colb[:], 1.0)
            ones_row16 = glob.tile([1, 128], FP16)
            nc.vector.memset(ones_row16[:], 1.0)
            eps_t = glob.tile([1, 1], F32)
            nc.vector.memset(eps_t[:], EPS)
            bq_t = glob.tile([128, HPG], F32)
            nc.sync.dma_start(out=bq_t, in_=bqd[:, :].rearrange("h d -> d h"))
            bk_t = glob.tile([128, HPG], F32)
            nc.sync.dma_start(out=bk_t, in_=bkd[:, :].rearrange("h d -> d h"))
            bv_t = glob.tile([1, GF], FP16)
            nc.sync.dma_start(out=bv_t, in_=bvd[:, :])
            ropeq_a = glob.tile([128, S], FP16)
            ropeq_b = glob.tile([128, S], FP16)
            ropek_a = glob.tile([128, S], FP16)
            ropek_b = glob.tile([128, S], FP16)

            q_sb = glob.tile([128, HPG, S], FP16)
            k_sb = glob.tile([128, HPG, S], FP16)
            v_sb = glob.tile([128, N_SK, GF], BF16)
            rkT = glob.tile([128, HPG, N_SK], F32)
            wo_s = glob.tile([128, HPG, C], BF16)
            oT = [glob.tile([128, HPG, SQ_T], BF16, name=f"oT{i}")
                  for i in range(N_SQ)]

            # ---------------- Phase 1: QKV + RMSNorm + RoPE ----------------
            with (
                tc.tile_pool(name="p1w", bufs=1) as p1w,
                tc.tile_pool(name="p1x", bufs=2) as p1x,
                tc.tile_pool(name="p1t", bufs=2) as p1t,
                tc.tile_pool(name="p1ps", bufs=4, space="PSUM") as p1ps,
                tc.tile_pool(name="p1psv", bufs=2, space="PSUM") as p1psv,
                tc.tile_pool(name="p1pss", bufs=2, space="PSUM") as p1pss,
            ):
                wk_s = p1w.tile([128, NCC, GF], FP16)
                wq_s = p1w.tile([128, NCC, GF], FP16)
                wv_s = p1w.tile([128, NCC, GF], FP16)
                # two chunks per weight: compute starts after the first half
                HC = NCC // 2
                for lo in (0, HC):
                    cs = slice(lo, lo + HC)
                    nc.sync.dma_start(out=wk_s[:, cs, :], in_=wk_r[:, cs, :])
                    nc.sync.dma_start(out=wv_s[:, cs, :], in_=wv_r[:, cs, :])
                    nc.sync.dma_start(out=wq_s[:, cs, :], in_=wq_r[:, cs, :])

                nc.sync.dma_start(out=ropek_a, in_=ropekd[0])
                nc.sync.dma_start(out=ropek_b, in_=ropekd[1])
                nc.sync.dma_start(out=ropeq_a, in_=ropeqd[0])
                nc.sync.dma_start(out=ropeq_b, in_=ropeqd[1])
                # wo needed only in phase 3
                for h in range(HPG):
                    nc.sync.dma_start(out=wo_s[:, h, :], in_=wo_r[:, h, :])

                for sweep, st in [(a, b) for a in (0, 1) for b in range(N_ST)]:
                    sl = slice(st * ST, (st + 1) * ST)
                    xs = p1x.tile([128, NCC, ST], FP16, tag="xs",
                                  name=f"xs{sweep}_{st}")
                    nc.scalar.dma_start(out=xs[:], in_=xt_r[:, :, sl])

                    for w_s, ra, rb, b_t, is_q in (
                        ((wk_s, ropek_a, ropek_b, bk_t, False),)
                        if sweep == 0 else
                        ((wq_s, ropeq_a, ropeq_b, bq_t, True),)
                    ):
                        for h in range(HPG):
                            hsl = slice(h * HD, (h + 1) * HD)
                            ps = p1ps.tile([128, ST], F32, tag="ps")
                            for cc in range(NCC):
                                nc.tensor.matmul(
                                    ps[:], w_s[:, cc, hsl], xs[:, cc, :],
                                    start=(cc == 0), stop=(cc == NCC - 1))
                            raw = p1t.tile([128, ST], FP16, tag="raw")
                            nc.scalar.activation(raw[:], ps[:], AF.Identity,
                                                 bias=b_t[:, h:h + 1])
                            sq2 = p1t.tile([128, ST], FP16, tag="sq2")
                            nc.vector.tensor_mul(sq2[:], raw[:], raw[:])
                            ssq = p1pss.tile([1, ST], F32, tag="ssq")
                            nc.tensor.matmul(ssq[:], ones_col16[:], sq2[:])
                            # rs = 1/sqrt(ms + eps): ACT Sqrt + DVE recip
                            sms = p1t.tile([1, ST], F32, tag="sms", bufs=1)
                            nc.scalar.activation(sms[:], ssq[:], AF.Sqrt,
                                                 scale=1.0 / HD,
                                                 bias=eps_t[:])
                            rsq = p1t.tile([1, ST], F32, tag="rsq")
                            nc.vector.reciprocal_approx_fast(out=rsq[:],
                                                             in_=sms[:])
                            if is_q:
                                rqst = p1t.tile([1, ST], FP16, tag="rqst")
                                nc.vector.tensor_copy(rqst[:], rsq[:])
                            else:
                                # rk row -> DRAM; transposed reload below
                                nc.gpsimd.dma_start(out=rkscr[h:h + 1, sl],
                                                    in_=rsq[:])
                            # rope: out_lo = ta_lo + ta_hi ; out_hi = tb_hi + tb_lo
                            ta = p1t.tile([128, ST], FP16, tag="ta")
                            tb = p1t.tile([128, ST], FP16, tag="tb")
                            nc.vector.tensor_mul(ta[:], ra[:, sl], raw[:])
                            nc.vector.tensor_mul(tb[:], rb[:, sl], raw[:])
                            m1 = p1t.tile([128, ST], FP16, tag="m1")
                            nc.gpsimd.dma_start(out=m1[0:64, :],
                                                in_=ta[64:128, :])
                            nc.gpsimd.dma_start(out=m1[64:128, :],
                                                in_=tb[0:64, :])
                            if is_q:
                                qt = p1t.tile([128, ST], FP16, tag="qt")
                                nc.vector.tensor_add(qt[0:64, :], ta[0:64, :],
                                                     m1[0:64, :])
                                nc.vector.tensor_add(qt[64:128, :],
                                                     tb[64:128, :],
                                                     m1[64:128, :])
                                rqb = p1t.tile([128, ST], FP16, tag="rqb")
                                nc.gpsimd.partition_broadcast(rqb[:], rqst[:])
                                nc.vector.tensor_mul(q_sb[:, h, sl], qt[:],
                                                     rqb[:])
                            else:
                                nc.vector.tensor_add(k_sb[0:64, h, sl],
                                                     ta[0:64, :], m1[0:64, :])
                                nc.vector.tensor_add(k_sb[64:128, h, sl],
                                                     tb[64:128, :],
                                                     m1[64:128, :])

                    for j in range(ST // 128 if sweep == 0 else 0):
                        vp = p1psv.tile([128, GF], F32, tag="vp")
                        jsl = slice(j * 128, (j + 1) * 128)
                        for cc in range(NCC):
                            nc.tensor.matmul(vp[:], xs[:, cc, jsl],
                                             wv_s[:, cc, :],
                                             start=(cc == 0), stop=False)
                        nc.tensor.matmul(vp[:], ones_row16[:], bv_t[:],
                                         start=False, stop=True)
                        nc.scalar.activation(
                            v_sb[:, st * (ST // 128) + j, :], vp[:], AF.Copy)

                    if sweep == 0 and st == N_ST - 1:
                        # k norm scales -> [key-part, chunk] via DRAM bounce
                        for h in range(HPG):
                            nc.gpsimd.dma_start(
                                out=rkT[:, h, :],
                                in_=rkscr[h:h + 1, :].rearrange(
                                    "o (c p) -> p (c o)", p=128))

            # -------- Phase 2 + 3: attention + output projection --------
            with (
                tc.tile_pool(name="p2e", bufs=2) as p2e,
                tc.tile_pool(name="p2t", bufs=2) as p2t,
                tc.tile_pool(name="p3t", bufs=3) as p3t,
                tc.tile_pool(name="p2sc", bufs=2, space="PSUM") as p2sc,
                tc.tile_pool(name="p2z", bufs=1, space="PSUM") as p2z,
                tc.tile_pool(name="p2o", bufs=1, space="PSUM") as p2o,
                tc.tile_pool(name="p3ps", bufs=1, space="PSUM") as p3ps,
            ):
                NJQ = SQ_T // 512  # 512-wide query sub-blocks
                for sq in range(N_SQ):
                    q0 = sq * SQ_T
                    for h in range(HPG):
                        es = p2e.tile([128, N_SK, SQ_T], BF16, tag="es")
                        for sk in range(N_SK):
                            ksl = slice(sk * 128, (sk + 1) * 128)
                            scj = p2sc.tile([128, SQ_T], F32, tag="sc")
                            for j in range(NJQ):
                                nc.tensor.matmul(
                                    scj[:, j * 512:(j + 1) * 512],
                                    k_sb[:, h, ksl],
                                    q_sb[:, h, q0 + j * 512:q0 + (j + 1) * 512])
                            nc.scalar.activation(
                                es[:, sk, :], scj[:],
                                AF.Exp, scale=rkT[:, h, sk:sk + 1])
                        # one z bank per query half: both accumulation
                        # chains run concurrently with the exp pipeline
                        rz = p2t.tile([1, SQ_T], F32, tag="rz")
                        rz16 = p2t.tile([1, SQ_T], BF16, tag="rz16")
                        rzb = p2t.tile([128, NJQ, 512], BF16, tag="rzb")
                        for j in range(NJQ):
                            jq = slice(j * 512, (j + 1) * 512)
                            z_ps = p2z.tile([1, 512], F32, tag=f"z{j}")
                            for sk in range(N_SK):
                                nc.tensor.matmul(
                                    z_ps[0:1, :], ones_colb[:],
                                    es[:, sk, jq],
                                    start=(sk == 0), stop=(sk == N_SK - 1))
                            nc.vector.reciprocal_approx_fast(
                                out=rz[0:1, jq], in_=z_ps[0:1, :])
                            nc.vector.tensor_copy(rz16[0:1, jq], rz[0:1, jq])
                            nc.gpsimd.partition_broadcast(rzb[:, j, :],
                                                          rz16[0:1, jq])
                        for j in range(NJQ):
                            o_ps = p2o.tile([128, 512], F32, tag="o")
                            jq = slice(j * 512, (j + 1) * 512)
                            for sk in range(N_SK):
                                nc.tensor.matmul(
                                    o_ps[:], v_sb[:, sk, h * HD:(h + 1) * HD],
                                    es[:, sk, jq],
                                    start=(sk == 0), stop=(sk == N_SK - 1))
                            nc.vector.scalar_tensor_tensor(
                                out=oT[sq][:, h, jq], in0=o_ps[:], scalar=1.0,
                                in1=rzb[:, j, :], op0=ALU.mult, op1=ALU.mult)

                    # phase 3 for this query block
                    for r in range(SQ_T // 128):
                        rsl = slice(q0 + r * 128, q0 + (r + 1) * 128)
                        orl = slice(r * 128, (r + 1) * 128)
                        for j in range(C // 512):
                            jsl = slice(j * 512, (j + 1) * 512)
                            op = p3ps.tile([128, 512], F32, tag="op")
                            for h in range(HPG):
                                nc.tensor.matmul(op[:], oT[sq][:, h, orl],
                                                 wo_s[:, h, jsl],
                                                 start=(h == 0),
                                                 stop=(h == HPG - 1))
                            oe3 = p3t.tile([128, 512], F32, tag="oe3")
                            nc.vector.tensor_copy(oe3[:], op[:])
                            nc.sync.dma_start(out=outd[rsl, jsl], in_=oe3[:])

    nc.compile()
    return nc


_PROGRAM = None


def _get_program():
    global _PROGRAM
    if _PROGRAM is None:
        _PROGRAM = build_program()
    return _PROGRAM


def _perm128():
    # even head dims then odd head dims
    return np.concatenate([np.arange(0, HD, 2), np.arange(1, HD, 2)])


def _rope_tiles(rotary_emb, norm_w, extra_scale):
    """Fold per-dim norm weight (and optional score scale) into rope coeffs.

    Returns [2, 128, S]: [a/b, partition, s] where partitions 0:64 multiply
    the even input dims and 64:128 the odd input dims.
    """
    f32 = np.float32
    Rt = np.asarray(rotary_emb, f32)[0, :, 0].transpose(1, 2, 3, 0)  # [64,2,2,S]
    w = np.asarray(norm_w, f32) * extra_scale
    we = w[0::2][:, None]
    wo = w[1::2][:, None]
    out = np.empty((2, 128, S), f32)
    out[0, 0:64] = Rt[:, 0, 0, :] * we
    out[0, 64:128] = Rt[:, 0, 1, :] * wo
    out[1, 0:64] = Rt[:, 1, 0, :] * we
    out[1, 64:128] = Rt[:, 1, 1, :] * wo
    return out


def prepare_in_maps(hidden_states, rotary_emb, wq, bq, wk, bk, wv, bv,
                    q_norm_w, k_norm_w, wo, bo):
    f32 = np.float32
    fp16 = np.float16
    bf16 = ml_dtypes.bfloat16
    hidden_states = np.asarray(hidden_states, f32)
    wq, bq = np.asarray(wq, f32), np.asarray(bq, f32)
    wk, bk = np.asarray(wk, f32), np.asarray(bk, f32)
    wv, bv = np.asarray(wv, f32), np.asarray(bv, f32)
    wo = np.asarray(wo, f32)

    p128 = _perm128()
    ropeq = _rope_tiles(rotary_emb, q_norm_w, ISCALE).astype(fp16)
    ropek = _rope_tiles(rotary_emb, k_norm_w, 1.0).astype(fp16)

    wqT = wq.T  # [in C, out C]
    wkT = wk.T
    wvT = wv.T
    woT = wo.T  # [d, j]

    in_maps = []
    for core in range(N_CORES):
        b, g = divmod(core, HG)
        base = g * GF
        cols = np.concatenate(
            [base + hh * HD + p128 for hh in range(HPG)])
        xtb = np.ascontiguousarray(hidden_states[b].T).astype(fp16)
        in_maps.append({
            "xt": xtb,
            "wqt": np.ascontiguousarray(wqT[:, cols]).astype(fp16),
            "wkt": np.ascontiguousarray(wkT[:, cols]).astype(fp16),
            "wvt": np.ascontiguousarray(wvT[:, base:base + GF]).astype(fp16),
            "wot": np.ascontiguousarray(woT[base:base + GF, :]).astype(bf16),
            "bq": np.ascontiguousarray(bq[cols]).reshape(HPG, HD),
            "bk": np.ascontiguousarray(bk[cols]).reshape(HPG, HD),
            "bv": np.ascontiguousarray(bv[base:base + GF])[None, :].astype(fp16),
            "ropeq": ropeq,
            "ropek": ropek,
        })
    return in_maps


def combine_results(results, bo):
    bo = np.asarray(bo, np.float32)
    out = np.zeros((B, S, C), np.float32)
    for core in range(N_CORES):
        b = core // HG
        out[b] += results[core]["out"]
    out += bo
    return out


def kernel(hidden_states, rotary_emb, wq, bq, wk, bk, wv, bv,
           q_norm_w, k_norm_w, wo, bo):
    nc = _get_program()
    in_maps = prepare_in_maps(hidden_states, rotary_emb, wq, bq, wk, bk,
                              wv, bv, q_norm_w, k_norm_w, wo, bo)
    res = run_bass_kernel_spmd(nc, in_maps, list(range(N_CORES)))
    return combine_results(res.results, bo)
